# revision 52
# baseline (speedup 1.0000x reference)
"""Bayesian linear layer (reparameterized) on 8 Trainium2 NeuronCores.

y = x @ (mu + exp(log_sigma) * eps_w).T + (bias_mu + exp(bias_log_sigma) * eps_b)

Shapes: x [8192, 4096] f32, weights [16384, 4096] f32, y [8192, 16384] f32.

Strategy (column-parallel / tensor-parallel over out_features):
  - Shard all [OUT, IN] weight tensors and the bias vectors along OUT across
    8 cores (2048 out features per core); replicate x.
  - Host feeds x.T (bf16) and W-shard transposes [IN, OUT_S] so the
    contraction dim lands on SBUF partitions with natural (contiguous) DMAs.
  - On device: build W.T in bf16, resident in SBUF; 32x accumulating
    bf16 matmuls per [128 tok x 512 out] tile into fp32 PSUM; add the
    (replicated) bias during PSUM eviction on the vector engine.
  - Host concatenates the per-core [8192, 2048] f32 outputs along axis 1.
"""

import os
import sys

sys.path.insert(0, "/opt/trn_rl_repo")
os.environ.setdefault("MYCRO_LOCAL_CACHE", "1")

import numpy as np
import ml_dtypes

N_TOK, IN_DIM, OUT_DIM = 8192, 4096, 16384
N_CORES = 8
OUT_S = OUT_DIM // N_CORES  # 2048
P = 128


def build_program(n_tok=N_TOK, in_dim=IN_DIM, out_s=OUT_S, n_cores=N_CORES,
                  chunk=512, xt_bufs=3, out_bufs=4, psum_bufs=8, scratch_bufs=3):
    """Build + compile the single-core Bass program (SPMD across cores)."""
    import concourse.bass as bass
    import concourse.mybir as mybir
    import concourse.tile as tile
    from concourse import bacc
    from contextlib import ExitStack

    fp32 = mybir.dt.float32
    bf16 = mybir.dt.bfloat16
    fp16 = mybir.dt.float16
    Exp = mybir.ActivationFunctionType.Exp
    add = mybir.AluOpType.add

    KT = in_dim // P        # k tiles (contraction)
    MT = n_tok // P         # token tiles
    NO = out_s // 512       # psum-width output chunks
    NCH = out_s // chunk    # elementwise build chunks
    assert in_dim % P == 0 and n_tok % P == 0 and out_s % 512 == 0 and out_s % chunk == 0

    nc = bacc.Bacc("TRN2", target_bir_lowering=False, debug=False,
                   num_devices=n_cores, enable_asserts=False)

    # x pre-tiled on host: xB[m, ki, ko, t] = x[m*128 + t, ko*128 + ki],
    # so each token-tile load is one fully contiguous DMA.
    xB = nc.dram_tensor("xB", [n_tok // P, P, in_dim // P, P], bf16,
                        kind="ExternalInput")
    # fp16 (not bf16): ls ~ -5, and bf16's 8-bit mantissa on ls is a ~1%
    # multiplicative error after exp; fp16's 10 bits keep the whole pipeline
    # at f32-input accuracy (verified numerically) at half the DMA traffic.
    muT = nc.dram_tensor("muT", [in_dim, out_s], fp16, kind="ExternalInput")
    lsT = nc.dram_tensor("lsT", [in_dim, out_s], fp16, kind="ExternalInput")
    epsT = nc.dram_tensor("epsT", [in_dim, out_s], fp16, kind="ExternalInput")
    bmu = nc.dram_tensor("bmu", [out_s], fp32, kind="ExternalInput")
    bls = nc.dram_tensor("bls", [out_s], fp32, kind="ExternalInput")
    beps = nc.dram_tensor("beps", [out_s], fp32, kind="ExternalInput")
    y = nc.dram_tensor("y", [n_tok, out_s], fp32, kind="ExternalOutput")

    with tile.TileContext(nc) as tc, ExitStack() as ctx:
        wt_pool = ctx.enter_context(tc.tile_pool(name="wt", bufs=1))
        const_pool = ctx.enter_context(tc.tile_pool(name="const", bufs=1))
        scratch = ctx.enter_context(tc.tile_pool(name="scratch", bufs=scratch_bufs))
        xt_pool = ctx.enter_context(tc.tile_pool(name="xt", bufs=xt_bufs))
        out_pool = ctx.enter_context(tc.tile_pool(name="out", bufs=out_bufs))
        psum_pool = ctx.enter_context(
            tc.tile_pool(name="psum", bufs=psum_bufs, space="PSUM"))

        def fused_w(dst_ap, ls_src, eps_src, mu_src, sync_engine, dt_in,
                    pre="", bufs=None, width=None, ew_engine=None,
                    exp_dt=None):
            # dst = mu + exp(ls) * eps, elementwise over a [P, width] block
            w = chunk if width is None else width
            kw = {} if bufs is None else {"bufs": bufs}
            l = scratch.tile([P, w], dt_in, tag=pre + "ls", name="ls_t", **kw)
            e = scratch.tile([P, w], dt_in, tag=pre + "eps", name="eps_t", **kw)
            m_ = scratch.tile([P, w], dt_in, tag=pre + "mu", name="mu_t", **kw)
            x_ = scratch.tile([P, w], exp_dt or fp32, tag=pre + "exp",
                              name="exp_t", **kw)
            sync_engine.dma_start(out=l[:], in_=ls_src)
            sync_engine.dma_start(out=e[:], in_=eps_src)
            sync_engine.dma_start(out=m_[:], in_=mu_src)
            ew = ew_engine or nc.vector
            nc.scalar.activation(x_[:], l[:], Exp)
            ew.tensor_mul(x_[:], x_[:], e[:])
            ew.tensor_tensor(dst_ap, x_[:], m_[:], add)

        # bias_rep[p, o] = bmu[o] + exp(bls[o]) * beps[o]; bf16 is plenty
        # (it is added into the f32 psum at eviction).
        bias_rep = const_pool.tile([P, out_s], bf16, tag="bias_rep",
                                   name="bias_rep")

        def bias_chunk(j):
            sl = slice(j * chunk, (j + 1) * chunk)
            fused_w(bias_rep[:, sl],
                    bls.ap()[sl].partition_broadcast(P),
                    beps.ap()[sl].partition_broadcast(P),
                    bmu.ap()[sl].partition_broadcast(P),
                    nc.gpsimd, fp32, pre="b", bufs=1)

        # ---- W build + matmul, grouped by 1024-wide output column pairs ----
        # 1024-wide W tiles give 2 KB-per-partition DMA lines (half the
        # descriptor count of 512-wide ones). Group 0 (first pair) builds
        # first and its full token sweep starts immediately; the second
        # pair's weight inputs stream in behind that compute. W.T stays
        # SBUF-resident in bf16.
        OCW = 1024  # W-tile width; each holds 2 psum-width (512) columns
        assert out_s % OCW == 0
        NP = out_s // OCW
        groups = [[p] for p in range(NP)]

        wt = {}  # (k, p) -> [P, OCW] bf16 tile

        def build_w_chunk(k, p):
            t = wt_pool.tile([P, OCW], bf16, tag=f"wt{k}_{p}",
                             name=f"wt{k}_{p}")
            wt[(k, p)] = t
            rows = slice(k * P, (k + 1) * P)
            sl = slice(p * OCW, (p + 1) * OCW)
            # Alternate the elementwise mul/add between DVE and GpSimd: the
            # build is elementwise-throughput-bound during the first group's
            # window, and GpSimd is otherwise idle. bf16 exp output halves
            # the elementwise bytes (precision is bounded by the final bf16
            # W anyway).
            ew = nc.vector if k % 2 == 0 else nc.gpsimd
            fused_w(t[:],
                    lsT.ap()[rows, sl],
                    epsT.ap()[rows, sl],
                    muT.ap()[rows, sl],
                    nc.sync, fp16, width=OCW, bufs=2, ew_engine=ew,
                    exp_dt=bf16)

        def load_xt(m):
            xt = xt_pool.tile([P, KT, P], bf16, tag="xt", name="xt")
            nc.sync.dma_start(out=xt[:], in_=xB.ap()[m])
            return xt

        # Stagger the first token-tile prefetches between the first W-build
        # chunks so neither stream delays the other at kernel start.
        xt_ahead = []

        for gi, g in enumerate(groups):
            if gi == 0:
                xt_ahead.append(load_xt(0))
                # Warm-up: throwaway matmuls with no W dependency keep the
                # PE dense through the W-build window, so the HAM clock gate
                # opens to 8/8 once and stays (idle >3.4us re-throttles to
                # half clock).
                warm_ps = psum_pool.tile([P, 512], fp32, tag="ps",
                                         name="warm_ps")
                for _ in range(60):
                    nc.tensor.matmul(warm_ps[:, :P], xt_ahead[0][:, 0, :],
                                     xt_ahead[0][:, 1, :],
                                     start=True, stop=True)
                while len(xt_ahead) < min(xt_bufs, MT):
                    xt_ahead.append(load_xt(len(xt_ahead)))
                for k in range(KT):
                    for p in g:
                        build_w_chunk(k, p)
                for oc in range(NO):
                    bias_chunk(oc)
            # Next group's bias/W-build chunks are interleaved into this
            # group's m-loop below so their DMA/ACT/DVE work overlaps matmul
            # compute instead of queueing behind the whole group in program
            # order.
            nxt = groups[gi + 1] if gi + 1 < len(groups) else []
            pending = [(lambda k=k, p=p: build_w_chunk(k, p))
                       for k in range(KT) for p in nxt]
            n_pending = len(pending)
            pending = iter(pending)
            ocs = [p * 2 + j for p in g for j in range(2)]  # 512-wide cols

            def evict(psums, m):
                for oc in ocs:
                    ot = out_pool.tile([P, 512], fp32, tag="ot", name="ot")
                    nc.vector.tensor_tensor(ot[:], psums[oc][:],
                                            bias_rep[:, oc * 512:(oc + 1) * 512],
                                            add)
                    # SWDGE (gpsimd): y stores wait on the eviction, and on
                    # the sync stream that wait head-of-line-blocks the next
                    # x-tile load; stores are latency-insensitive, so keep
                    # them off the load queues entirely.
                    nc.gpsimd.dma_start(
                        out=y.ap()[m * P:(m + 1) * P, oc * 512:(oc + 1) * 512],
                        in_=ot[:])

            def alloc_psums(m):
                return {oc: psum_pool.tile([P, 512], fp32, tag="ps",
                                           name=f"ps{m}_{oc}")
                        for oc in ocs}

            per_iter = -(-n_pending // max(MT - 8, 1))
            for m in range(MT):
                if xt_ahead:
                    xt = xt_ahead.pop(0)
                else:
                    xt = load_xt(m)

                for _ in range(per_iter):
                    job = next(pending, None)
                    if job is not None:
                        job()

                psums = alloc_psums(m)
                for k in range(KT):
                    lhsT = xt[:, k, :]
                    for p in g:
                        for j in range(2):
                            nc.tensor.matmul(
                                psums[p * 2 + j][:], lhsT,
                                wt[(k, p)][:, j * 512:(j + 1) * 512],
                                start=(k == 0), stop=(k == KT - 1))
                evict(psums, m)
            for job in pending:
                job()

    nc.compile()
    return nc


_PROGRAM_CACHE = {}


def _get_program():
    key = (N_TOK, IN_DIM, OUT_S)
    if key not in _PROGRAM_CACHE:
        _PROGRAM_CACHE[key] = build_program()
    return _PROGRAM_CACHE[key]


def make_in_maps(x, weight_mu, weight_log_sigma, bias_mu, bias_log_sigma,
                 eps_w, eps_b):
    x = np.asarray(x, dtype=np.float32)
    weight_mu = np.asarray(weight_mu, dtype=np.float32)
    weight_log_sigma = np.asarray(weight_log_sigma, dtype=np.float32)
    bias_mu = np.asarray(bias_mu, dtype=np.float32)
    bias_log_sigma = np.asarray(bias_log_sigma, dtype=np.float32)
    eps_w = np.asarray(eps_w, dtype=np.float32)
    eps_b = np.asarray(eps_b, dtype=np.float32)

    # xB[m, ki, ko, t] = x[m*128 + t, ko*128 + ki]
    MT, KT = N_TOK // P, IN_DIM // P
    xB = x.reshape(MT, P, KT, P).transpose(0, 3, 2, 1).astype(ml_dtypes.bfloat16)
    in_maps = []
    for c in range(N_CORES):
        sl = slice(c * OUT_S, (c + 1) * OUT_S)
        in_maps.append({
            "xB": xB,
            "muT": weight_mu[sl].T.astype(np.float16),
            "lsT": weight_log_sigma[sl].T.astype(np.float16),
            "epsT": eps_w[sl].T.astype(np.float16),
            "bmu": np.ascontiguousarray(bias_mu[sl]),
            "bls": np.ascontiguousarray(bias_log_sigma[sl]),
            "beps": np.ascontiguousarray(eps_b[sl]),
        })
    return in_maps


def run(in_maps, trace=False, **kwargs):
    from concourse.bass_utils import run_bass_kernel_spmd
    nc = _get_program()
    res = run_bass_kernel_spmd(nc, in_maps, list(range(N_CORES)),
                               trace=trace, **kwargs)
    out = np.concatenate([res.results[c]["y"] for c in range(N_CORES)], axis=1)
    return out, res


def kernel(x, weight_mu, weight_log_sigma, bias_mu, bias_log_sigma,
           eps_w, eps_b):
    in_maps = make_in_maps(x, weight_mu, weight_log_sigma, bias_mu,
                           bias_log_sigma, eps_w, eps_b)
    out, _ = run(in_maps, trace=False)
    return out
